# revision 2
# baseline (speedup 1.0000x reference)
"""Causal scaled-dot-product attention on 8 Trainium2 NeuronCores.

Problem: B=2, H=16, S=2048, D=64, fp32, causal mask.
Sharding: batch*heads (32) split 4-per-core across 8 cores; no collectives.

Per-core bass/Tile kernel (v2 — exp split across engines):

Phase 1 (per k-chunk row ci, head pair A/B stacked on SBUF partitions
0-63 / 64-127):
  - S^T[k, q] = (K^T)^T @ Q^T on PE fp16, with tile_position row tiling
    (64x128 mode, tiles T0/T8) so the two heads' matmuls run CONCURRENTLY.
  - exp: the 128-wide diagonal piece always uses exact ScalarE table exp
    (self-attention keys dominate softmax mass); off-diagonal 512-wide
    pieces are load-balanced between ScalarE (exact exp) and the DVE via
    a one-op Schraudolph trick: int16(round(s*log2e*1024 + 15*1024))
    bit-cast as fp16 IS 2^(s*log2e) to ~3%% — errors cancel through the
    softmax normalization since l sums the same approximated values.
  - Diagonal 128x128 tile: GPSIMD affine_select zeroes P^T where k > q.

Phase 2 (lagging LAG rows): for q-tile qt, all 4 heads' O accumulate in
ONE PSUM bank ([128, 4, 65]; V_aug = [V | 1] provides l in column 64),
evacuated by ScalarE-Copy/DVE-copy (balance-picked) into an SBUF stage
and DMA'd out raw. Host does O = O'/l (normalization off-device).
"""

import sys
import numpy as np
from contextlib import ExitStack

B, H, S, D = 2, 16, 2048, 64
N_CORES = 8
HEADS_PER_CORE = (B * H) // N_CORES  # 4
CH = 128             # k-chunk (partition tile)
PIECE_W = 512        # off-diag S^T piece width (1 PSUM bank per head)
DIAG_W = 128         # diagonal piece width (always exact ScalarE exp)
SCALE = 1.0 / np.sqrt(D)
MM_DTYPE = "float16"
# Schraudolph exp-as-bitcast constants (fp16 layout: 5 exp bits bias 15,
# 10 mantissa bits). bits = s*SCALE*log2e*1024 + (15+C)*1024.
SCH_A = float(SCALE * np.log2(np.e) * 1024.0)
SCH_C = -0.058
SCH_B = float((15.0 + SCH_C) * 1024.0)
# engine cost models for static load balancing (ns)
_SC_COST = lambda fd: (172.0 + fd) / 1.2
_VE_COST = lambda fd: (120.0 + fd) / 0.96 * 1.30

_NP_MM = {"float16": np.float16, "float32": np.float32}

for _p in ("/opt/trn_rl_repo", "/opt/pypackages"):
    if _p not in sys.path:
        sys.path.append(_p)


def _row_off(ci, s_len):
    # packed column offset of causal row ci: sum_{j<ci} (s_len - 128*j)
    return s_len * ci - CH * (ci * (ci - 1)) // 2


def _build_program(n_heads, s_len, piece_w=PIECE_W, mm_dtype=MM_DTYPE):
    import concourse.bass as bass  # noqa: F401
    import concourse.bacc as bacc
    import concourse.tile as tile
    from concourse import mybir

    f32 = mybir.dt.float32
    i16 = mybir.dt.int16
    mmdt = getattr(mybir.dt, mm_dtype)
    n_chunks = s_len // CH
    n_pairs = (n_heads + 1) // 2
    DP1 = D + 1
    pt_len = _row_off(n_chunks, s_len)  # packed P^T length per head

    nc = bacc.Bacc(
        "TRN2",
        target_bir_lowering=False,
        debug=False,
        num_devices=N_CORES,
    )

    qk_d = nc.dram_tensor("qk", [128, n_pairs, 2, s_len], mmdt, kind="ExternalInput").ap()
    v_d = nc.dram_tensor("v", [128, n_heads, n_chunks, DP1], mmdt, kind="ExternalInput").ap()
    # raw (unnormalized) output + l: per q-tile, [128 q, n_heads, D+1]
    o_d = nc.dram_tensor("o", [n_chunks, 128, n_heads * DP1], f32, kind="ExternalOutput").ap()

    # static greedy engine balance (ns accumulated per engine)
    bal = {"sc": 0.0, "ve": 0.0}

    def pick(fd, force=None):
        if force is None:
            force = "sc" if bal["sc"] + _SC_COST(fd) <= bal["ve"] + _VE_COST(fd) else "ve"
        bal[force] += _SC_COST(fd) if force == "sc" else _VE_COST(fd)
        return force

    with tile.TileContext(nc) as tc, ExitStack() as ctx:
        const = ctx.enter_context(tc.tile_pool(name="const", bufs=1))
        sb_pt = ctx.enter_context(tc.tile_pool(name="ptp", bufs=2))
        sb_st = ctx.enter_context(tc.tile_pool(name="stage", bufs=1))
        ps_s = ctx.enter_context(tc.tile_pool(name="pss", bufs=3, space="PSUM"))
        ps_o = ctx.enter_context(tc.tile_pool(name="pso", bufs=2, space="PSUM"))

        qk = const.tile([128, n_pairs, 2, s_len], mmdt)
        v = const.tile([128, n_heads, n_chunks, DP1], mmdt)
        for pair in range(n_pairs):
            nc.sync.dma_start(out=qk[:, pair], in_=qk_d[:, pair])
        for hh in range(n_heads):
            nc.sync.dma_start(out=v[:, hh], in_=v_d[:, hh])

        pair_heads = {p: [hh for hh in (2 * p, 2 * p + 1) if hh < n_heads]
                      for p in range(n_pairs)}
        pts = {p: sb_pt.tile([128, 2, pt_len], mmdt, tag="ptfull", name=f"ptp{p}")
               for p in range(n_pairs)}
        stage = sb_st.tile([128, n_chunks, n_heads * DP1], f32, name="ostage")

        def exp_piece(pt_pair, st, nh, ro, poff, w, eng):
            """exp of st[:, 0:nh, 0:w] (PSUM fp32) -> pt packed fp16."""
            if eng == "sc":
                nc.scalar.activation(
                    pt_pair[:, 0:nh, ro + poff:ro + poff + w],
                    st[:, 0:nh, 0:w],
                    mybir.ActivationFunctionType.Exp,
                    scale=float(SCALE),
                )
            else:
                nc.vector.tensor_scalar(
                    pt_pair[:, 0:nh, ro + poff:ro + poff + w].bitcast(i16),
                    st[:, 0:nh, 0:w],
                    SCH_A,
                    SCH_B,
                    mybir.AluOpType.mult,
                    mybir.AluOpType.add,
                )

        def ph1_row(pair, heads, ci, pt_pair):
            sp0 = CH * ci
            span = s_len - sp0
            ro = _row_off(ci, s_len)
            nh = len(heads)
            # pieces: [0, DIAG_W) exact-exp diagonal, then 512-wide pieces
            pieces = [(0, min(DIAG_W, span))]
            poff = DIAG_W
            while poff < span:
                pieces.append((poff, min(piece_w, span - poff)))
                poff += piece_w
            for pi, (poff, w) in enumerate(pieces):
                st = ps_s.tile([128, 2, piece_w], f32, tag="st")
                for idx, hh in enumerate(heads):
                    bp = 64 * (hh % 2)
                    nc.tensor.matmul(
                        st[:, idx, 0:w],
                        qk[bp:bp + 64, pair, 1, sp0:sp0 + CH],
                        qk[bp:bp + 64, pair, 0, sp0 + poff:sp0 + poff + w],
                        start=True,
                        stop=True,
                        tile_position=(bp, 0),
                    )
                eng = pick(nh * w, force="sc" if pi == 0 else None)
                exp_piece(pt_pair, st, nh, ro, poff, w, eng)
                if pi == 0:
                    for idx in range(nh):
                        nc.gpsimd.affine_select(
                            out=pt_pair[:, idx, ro:ro + CH],
                            in_=pt_pair[:, idx, ro:ro + CH],
                            compare_op=mybir.AluOpType.is_ge,
                            fill=0.0,
                            base=0,
                            pattern=[[1, CH]],
                            channel_multiplier=-1,
                        )

        def chain_quad(qt):
            """all heads' O[q-tile qt] accumulated into one PSUM bank,
            then raw (unnormalized) evac to stage + DMA out."""
            op = ps_o.tile([128, n_heads, DP1], f32, tag="op")
            for pair in range(n_pairs):
                for idx, hh in enumerate(pair_heads[pair]):
                    for ci in range(qt + 1):
                        sl = _row_off(ci, s_len) + CH * (qt - ci)
                        nc.tensor.matmul(
                            op[:, hh, :],
                            pts[pair][:, idx, sl:sl + CH],
                            v[:, hh, ci, :],
                            start=(ci == 0),
                            stop=(ci == qt),
                        )
            fd = n_heads * DP1
            eng = pick(fd)
            if eng == "sc":
                nc.scalar.activation(
                    stage[:, qt], op[:, :, :],
                    mybir.ActivationFunctionType.Copy,
                )
            else:
                nc.vector.tensor_copy(stage[:, qt], op[:, :, :])
            nc.sync.dma_start(out=o_d[qt], in_=stage[:, qt])

        LAG = 2
        pending = []
        for ci in range(n_chunks):
            for pair in range(n_pairs):
                ph1_row(pair, pair_heads[pair], ci, pts[pair])
            pending.append(ci)
            while len(pending) > LAG:
                chain_quad(pending.pop(0))
        while pending:
            chain_quad(pending.pop(0))

    nc.compile()
    return nc


_PROGRAM_CACHE = {}


def _get_program(n_heads=HEADS_PER_CORE, s_len=S, piece_w=PIECE_W, mm_dtype=MM_DTYPE):
    key = (n_heads, s_len, piece_w, mm_dtype)
    if key not in _PROGRAM_CACHE:
        _PROGRAM_CACHE[key] = _build_program(n_heads, s_len, piece_w, mm_dtype)
    return _PROGRAM_CACHE[key]


def _pack_core(Qf, Kf, Vf, heads, s_len=S, mm_dtype=MM_DTYPE):
    """Build the per-core input dict. Qf/Kf/Vf: [B*H, S, D] float32."""
    dt_np = _NP_MM[mm_dtype]
    n_heads = len(heads)
    n_pairs = (n_heads + 1) // 2
    n_chunks = s_len // CH
    qk = np.zeros((128, n_pairs, 2, s_len), dt_np)
    v = np.ones((128, n_heads, n_chunks, D + 1), dt_np)
    for i, hf in enumerate(heads):
        pair, side = divmod(i, 2)
        bp = 64 * side
        qk[bp:bp + 64, pair, 0] = Qf[hf].T
        qk[bp:bp + 64, pair, 1] = Kf[hf].T
        v[:, i, :, :D] = Vf[hf].reshape(n_chunks, CH, D).transpose(1, 0, 2)
    return {"qk": qk, "v": v}


def _unpack_core(o_np, s_len=S):
    """o_np: [n_chunks, 128, n_heads*(D+1)] raw -> [n_heads, S, D]."""
    n_chunks = s_len // CH
    o = o_np.reshape(n_chunks, 128, -1, D + 1)
    n_heads = o.shape[2]
    out = o[:, :, :, :D] / o[:, :, :, D:D + 1]
    # [qt, q128, h, d] -> [h, qt*128+q128, d]
    return out.transpose(2, 0, 1, 3).reshape(n_heads, s_len, D).astype(np.float32)


def kernel(Q, K, V, mask):
    Q = np.asarray(Q, np.float32)
    K = np.asarray(K, np.float32)
    V = np.asarray(V, np.float32)
    mask = np.asarray(mask)

    if not np.array_equal(mask, np.tril(np.ones((S, S), dtype=bool))):
        # Non-causal mask: not expected for this problem; numpy fallback.
        scores = np.einsum("bhqd,bhkd->bhqk", Q, K) * SCALE
        scores = np.where(mask, scores, -np.inf)
        scores -= scores.max(-1, keepdims=True)
        p = np.exp(scores)
        p /= p.sum(-1, keepdims=True)
        return np.einsum("bhqk,bhkd->bhqd", p, V).astype(np.float32)

    from concourse.bass_utils import run_bass_kernel_spmd

    Qf = Q.reshape(B * H, S, D)
    Kf = K.reshape(B * H, S, D)
    Vf = V.reshape(B * H, S, D)

    nc = _get_program()
    in_maps = [
        _pack_core(Qf, Kf, Vf, list(range(c * HEADS_PER_CORE, (c + 1) * HEADS_PER_CORE)))
        for c in range(N_CORES)
    ]
    res = run_bass_kernel_spmd(nc, in_maps, core_ids=list(range(N_CORES)))
    out = np.empty((B * H, S, D), np.float32)
    for c in range(N_CORES):
        out[c * HEADS_PER_CORE:(c + 1) * HEADS_PER_CORE] = _unpack_core(res.results[c]["o"])
    return out.reshape(B, H, S, D)
